# revision 11
# baseline (speedup 1.0000x reference)
"""Trainium2 Bass kernel for nn_EmbedLayer (gnn_message_passing).

Computes, for x:(B,T,A,NIN), wcf:(B,T,A,13):
  glob  = x[:,:,0,:]  @ W_glob.T + b_glob            (B,T,1024)
  loc   = x[:,:,1:,:] @ W_loc.T  + b_loc             (B,T,19,512)
  h1    = relu(perm_feat @ W_phys.T + b_phys)        (B,T,190,256)
  phys  = h1 @ W_phys3.T + b_phys3
  agent = segment-max of phys over the 19 pairs containing each agent

Strategy: data-parallel over the 1000 (B,T) frames across 8 NeuronCores
(125 frames/core).  On each core:
 - loc/glob: X rows are PE-transposed into [K,128]-tiles (fp32), then fp32r
   matmuls with X^T stationary and W^T (pre-transposed on host) streamed,
   accumulating over K=4096 into PSUM [125,512].  The transpose->copy->matmul
   chain is software-pipelined (transposes run SKEW groups ahead of their
   matmuls) and the DMA-heavy glob agent is spread across the schedule.
 - pair branch: U = wcf @ [W1|W2]^T is computed feature-on-partition, the
   190-pair expansion is done with broadcast adds over the lexicographic
   run structure of the pairs, phys3 runs feature-on-partition, and the
   per-agent max uses the run structure before a final PE transpose back
   to row-major.
"""

import os
import numpy as np
from contextlib import ExitStack

B, T, A, NIN = 10, 100, 20, 4096
NIMG, NATT, NPHYS = 1024, 512, 256
NCORES = 8
FTOT = B * T
F = FTOT // NCORES  # 125 frames per core
NPAIR = 190
P = 128
NKT = NIN // P  # 32 k-tiles

_CACHE = {}


def make_chunks(fh):
    """Pair-run work units.  Run i covers pairs (i, j>i): fh*(19-i) columns.
    Big runs are split by frame-range so the SBUF tile stays small; small
    tail runs are grouped so every chunk has >=256 (even) matmul columns.
    Returns a list of (entries, cols) where entries = [(i, f0, fw), ...].
    """
    MAXC = 640
    chunks = []
    cur, cols = [], 0
    for i in range(A - 1):
        w = A - 1 - i
        rc = fh * w
        if rc >= 256:
            # split this run alone into frame-parts
            if cur and cols >= 256:
                chunks.append((cur, cols))
                cur, cols = [], 0
            nparts = -(-rc // MAXC)
            base = fh // nparts
            rem = fh - base * nparts
            f0 = 0
            for k in range(nparts):
                fw = base + (1 if k < rem else 0)
                cur.append((i, f0, fw))
                cols += fw * w
                f0 += fw
                if cols >= 256:
                    chunks.append((cur, cols))
                    cur, cols = [], 0
        else:
            cur.append((i, 0, fh))
            cols += rc
            if cols >= 256:
                chunks.append((cur, cols))
                cur, cols = [], 0
    if cur:
        if chunks and cols < 256:
            pi, pc = chunks[-1]
            chunks[-1] = (pi + cur, pc + cols)
        else:
            chunks.append((cur, cols))
    return chunks


def subsizes(cols):
    # fp32r matmul requires even moving/dest innermost counts
    assert cols % 2 == 0, cols
    n = -(-cols // 512)
    base = (cols // n) & ~1
    extra = cols - base * n
    assert extra % 2 == 0
    out = [base] * n
    k = 0
    while extra > 0:
        out[k] += 2
        extra -= 2
        k = (k + 1) % n
    return out


def suboffs(cols):
    szs = subsizes(cols)
    offs, o = [], 0
    for s in szs:
        offs.append(o)
        o += s
    return list(zip(offs, szs))


def build_module(Fc=F):
    import concourse.tile as tile
    import concourse.mybir as mybir
    from concourse import bacc
    from concourse.masks import make_identity

    fp32 = mybir.dt.float32
    f32r = mybir.dt.float32r
    AF = mybir.ActivationFunctionType
    AX = mybir.AxisListType

    nc = bacc.Bacc("TRN2", target_bir_lowering=False, debug=False)

    x_d = nc.dram_tensor("x", [Fc, A, NIN], fp32, kind="ExternalInput")
    wcfT_d = nc.dram_tensor("wcfT", [13, Fc * A], f32r, kind="ExternalInput")
    wgT_d = nc.dram_tensor("wglobT", [NIN, NIMG], f32r, kind="ExternalInput")
    wlT_d = nc.dram_tensor("wlocT", [NIN, NATT], f32r, kind="ExternalInput")
    wcT_d = nc.dram_tensor("wcT", [13, 2 * NPHYS], f32r, kind="ExternalInput")
    w3T_d = nc.dram_tensor("w3T", [NPHYS, NPHYS], f32r, kind="ExternalInput")
    bg_d = nc.dram_tensor("b_glob", [NIMG], fp32, kind="ExternalInput")
    bl_d = nc.dram_tensor("b_loc", [NATT], fp32, kind="ExternalInput")
    bp_d = nc.dram_tensor("b_phys", [NPHYS], fp32, kind="ExternalInput")
    b3_d = nc.dram_tensor("b_phys3", [NPHYS], fp32, kind="ExternalInput")

    glob_d = nc.dram_tensor("glob", [Fc, NIMG], fp32, kind="ExternalOutput")
    loc_d = nc.dram_tensor("loc", [Fc, A - 1, NATT], fp32, kind="ExternalOutput")
    agent_d = nc.dram_tensor("agent", [Fc, A, NPHYS], fp32, kind="ExternalOutput")

    Fcp = min(Fc, P)
    halves = []
    f0 = 0
    for fh in (Fc - Fc // 2, Fc // 2):
        if fh:
            halves.append((f0, fh))
        f0 += fh
    fh_max = max(fh for _, fh in halves)
    ph_cols_max = max(cols + cols % 2
                      for _, fh in halves for _, cols in make_chunks(fh))

    KQ = 8            # k-tiles per xa block
    GPB = KQ // 2     # transpose groups per xa block
    BPA = NKT // KQ   # xa blocks per agent
    GPA = NKT // 2    # groups per agent
    SKEW = 3          # transpose groups emitted ahead of their matmuls

    with tile.TileContext(nc) as tc:
        with ExitStack() as ctx:
            const = ctx.enter_context(tc.tile_pool(name="const", bufs=1))
            wlp = ctx.enter_context(tc.tile_pool(name="wl", bufs=1))
            wgp = ctx.enter_context(tc.tile_pool(name="wg", bufs=3))
            xap = ctx.enter_context(tc.tile_pool(name="xa", bufs=5))
            xtp = ctx.enter_context(tc.tile_pool(name="xt", bufs=SKEW + 3))
            up = ctx.enter_context(tc.tile_pool(name="u", bufs=5))
            wcfp = ctx.enter_context(tc.tile_pool(name="wcf", bufs=1))
            php = ctx.enter_context(tc.tile_pool(name="ph", bufs=7))
            amp = ctx.enter_context(tc.tile_pool(name="am", bufs=3))
            tmpp = ctx.enter_context(tc.tile_pool(name="tmp", bufs=2))
            outp = ctx.enter_context(tc.tile_pool(name="out", bufs=2))
            goutp = ctx.enter_context(tc.tile_pool(name="gout", bufs=1))
            aoutp = ctx.enter_context(tc.tile_pool(name="aout", bufs=2))

            ps_t = ctx.enter_context(tc.tile_pool(name="ps_t", bufs=2, space="PSUM"))
            ps_mm = ctx.enter_context(tc.tile_pool(name="ps_mm", bufs=3, space="PSUM"))
            ps_ph = ctx.enter_context(tc.tile_pool(name="ps_ph", bufs=2, space="PSUM"))

            ident = const.tile([P, P], fp32)
            make_identity(nc, ident[:])

            # ---- small constants first (fast DMAs) ----
            wcT_sb = const.tile([13, 2 * NPHYS], f32r)
            nc.sync.dma_start(wcT_sb[:], wcT_d.ap())
            w3_sb = const.tile([P, 2, NPHYS], f32r)
            nc.sync.dma_start(w3_sb[:], w3T_d.ap().rearrange("(kt p) n -> p kt n", p=P))
            bp_sb = const.tile([P, 2], fp32)
            nc.sync.dma_start(bp_sb[:], bp_d.ap().rearrange("(b p) -> p b", p=P))
            b3_sb = const.tile([P, 2], fp32)
            nc.sync.dma_start(b3_sb[:], b3_d.ap().rearrange("(b p) -> p b", p=P))
            bl_bc = const.tile([P, NATT], fp32)
            nc.sync.dma_start(bl_bc[:1, :], bl_d.ap()[None, :])
            nc.gpsimd.partition_broadcast(bl_bc[:Fcp, :], bl_bc[:1, :])
            bg_bc = const.tile([P, NIMG], fp32)
            nc.sync.dma_start(bg_bc[:1, :], bg_d.ap()[None, :])
            nc.gpsimd.partition_broadcast(bg_bc[:Fcp, :], bg_bc[:1, :])

            # loc weights resident, loaded in 4 chunks on the SWDGE queue so
            # they don't serialize the SP HWDGE stream of xa loads
            wl_sb = wlp.tile([P, NKT, NATT], f32r)
            wlT_v = wlT_d.ap().rearrange("(kt p) n -> p kt n", p=P)
            for c in range(4):
                nc.gpsimd.dma_start(wl_sb[:, c * 8:(c + 1) * 8, :],
                                    wlT_v[:, c * 8:(c + 1) * 8, :])

            wgT_v = wgT_d.ap().rearrange("(kt p) n -> p kt n", p=P)
            agrows = agent_d.ap().rearrange("f a n -> (f a) n")

            u_tiles = {}
            am_tiles = {}

            def emit_u(h):
                hf0, fh = halves[h]
                wcf_sb = wcfp.tile([13, fh_max * A], f32r, tag="wcf")
                nc.sync.dma_start(wcf_sb[:, :fh * A],
                                  wcfT_d.ap()[:, hf0 * A:(hf0 + fh) * A])
                uts = []
                for m in range(4):
                    ut = up.tile([P, fh_max * A], fp32, tag="u")
                    uts.append(ut)
                    for so, sw in suboffs(fh * A):
                        pu = ps_ph.tile([P, 512], fp32, tag="psph")
                        nc.tensor.matmul(pu[:, :sw],
                                         wcT_sb[:, m * P:(m + 1) * P],
                                         wcf_sb[:, so:so + sw],
                                         start=True, stop=True)
                        if m < 2:
                            nc.scalar.activation(ut[:, so:so + sw], pu[:, :sw],
                                                 AF.Identity, bias=bp_sb[:, m:m + 1])
                        else:
                            nc.scalar.copy(ut[:, so:so + sw], pu[:, :sw])
                u_tiles[h] = uts
                amb = []
                for b in range(2):
                    am = amp.tile([P, fh_max * A], fp32, tag="am")
                    nc.gpsimd.memset(am[:], -1e30)
                    amb.append(am)
                am_tiles[h] = amb

            def emit_phys_chunk(h, entries, cols):
                hf0, fh = halves[h]
                uts = u_tiles[h]
                cols_pad = cols + (cols % 2)
                offs = [0]
                for (i, e0, ew) in entries:
                    offs.append(offs[-1] + ew * (A - 1 - i))
                h1s = []
                for b in range(2):
                    h1pre = php.tile([P, ph_cols_max], fp32, tag="ph")
                    uv0 = uts[b].rearrange("p (f a) -> p f a", a=A)
                    uv2 = uts[2 + b].rearrange("p (f a) -> p f a", a=A)
                    for r, (i, e0, ew) in enumerate(entries):
                        w = A - 1 - i
                        ov = h1pre[:, offs[r]:offs[r + 1]].rearrange(
                            "p (f q) -> p f q", q=w)
                        nc.gpsimd.tensor_add(
                            ov,
                            uv0[:, e0:e0 + ew, i:i + 1].to_broadcast((P, ew, w)),
                            uv2[:, e0:e0 + ew, i + 1:A])
                    if cols_pad != cols:
                        nc.gpsimd.memset(h1pre[:, cols:cols_pad], 0.0)
                    h1 = php.tile([P, ph_cols_max], f32r, tag="ph", name="h1")
                    nc.scalar.activation(h1[:, :cols_pad], h1pre[:, :cols_pad],
                                         AF.Relu)
                    h1s.append(h1)
                for b in range(2):
                    py = php.tile([P, ph_cols_max], fp32, tag="ph")
                    for so, sw in suboffs(cols_pad):
                        pp = ps_ph.tile([P, 512], fp32, tag="psph")
                        for kt in range(2):
                            nc.tensor.matmul(
                                pp[:, :sw],
                                w3_sb[:, kt, b * P:(b + 1) * P],
                                h1s[kt][:, so:so + sw],
                                start=(kt == 0), stop=(kt == 1))
                        nc.scalar.activation(py[:, so:so + sw], pp[:, :sw],
                                             AF.Identity, bias=b3_sb[:, b:b + 1])
                    amv = am_tiles[h][b].rearrange("p (f a) -> p f a", a=A)
                    for r, (i, e0, ew) in enumerate(entries):
                        w = A - 1 - i
                        pv = py[:, offs[r]:offs[r + 1]].rearrange(
                            "p (f q) -> p f q", q=w)
                        # pairs (i, j) update agents j = i+1..19
                        nc.vector.tensor_max(amv[:, e0:e0 + ew, i + 1:A],
                                             amv[:, e0:e0 + ew, i + 1:A], pv)
                        # max over the run updates agent i
                        tmp = tmpp.tile([P, P], fp32, tag="tmp")
                        nc.vector.reduce_max(tmp[:, :ew], pv, axis=AX.X)
                        nc.vector.tensor_max(amv[:, e0:e0 + ew, i],
                                             amv[:, e0:e0 + ew, i], tmp[:, :ew])

            def emit_agent_out(h):
                hf0, fh = halves[h]
                rows = fh * A
                r0 = 0
                while r0 < rows:
                    cw = min(125, rows - r0)
                    pst = ps_t.tile([P, 512], fp32, tag="pst")
                    for b in range(2):
                        nc.tensor.transpose(pst[:cw, b * 256:b * 256 + P],
                                            am_tiles[h][b][:, r0:r0 + cw],
                                            ident[:, :])
                    aout = aoutp.tile([125, NPHYS], fp32, tag="aout")
                    nc.scalar.copy(
                        aout[:cw, :],
                        pst.rearrange("p (b c) -> p b c", b=2, c=256)[:cw, :, :P])
                    nc.scalar.dma_start(agrows[hf0 * A + r0:hf0 * A + r0 + cw, :],
                                        aout[:cw, :])
                    r0 += cw

            # phys work queue: halves strictly sequential
            queue = []
            emit_u(0)
            for entries, cols in make_chunks(halves[0][1]):
                queue.append((emit_phys_chunk, (0, entries, cols)))

            def h0_to_h1():
                emit_agent_out(0)
                if len(halves) > 1:
                    emit_u(1)
            queue.append((h0_to_h1, ()))
            if len(halves) > 1:
                for entries, cols in make_chunks(halves[1][1]):
                    queue.append((emit_phys_chunk, (1, entries, cols)))
                queue.append((emit_agent_out, (1,)))
            n_q = len(queue)
            q_emitted = [0]

            def pump_queue(target):
                while q_emitted[0] < min(target, n_q):
                    fn, args = queue[q_emitted[0]]
                    fn(*args)
                    q_emitted[0] += 1

            # ---- loc/glob pipeline over a flat group list ----
            # group = (a, kt0): 2 k-tile transposes + 2 matmuls.
            # agent 0 (glob, 16MB of streamed weights) is spread through the
            # schedule one group per ~19 slots.
            loc_groups = [(a, g * 2) for a in range(1, A) for g in range(GPA)]
            glob_groups = [(0, g * 2) for g in range(GPA)]
            groups = []
            li = 0
            step = max(1, len(loc_groups) // len(glob_groups))
            for k, gg in enumerate(glob_groups):
                groups.append(gg)
                groups.extend(loc_groups[li:li + step])
                li += step
            groups.extend(loc_groups[li:])
            NG = len(groups)

            def blk(gi):
                a, kt0 = groups[gi]
                return (a, kt0 // KQ)

            xa_tiles = {}
            xa_issued = set()
            xt_tiles = {}
            wg_tiles = {}
            wg_issued = set()
            psout = {}
            agent_done = {}

            def issue_xa(key):
                if key in xa_issued:
                    return
                xa_issued.add(key)
                a, q = key
                t = xap.tile([Fcp, KQ * P], fp32, tag="xa", name="xat")
                nc.sync.dma_start(t[:Fc, :],
                                  x_d.ap()[:, a, q * KQ * P:(q + 1) * KQ * P])
                xa_tiles[key] = t

            def issue_wg(gi):
                if gi in wg_issued:
                    return
                wg_issued.add(gi)
                a, kt0 = groups[gi]
                t = wgp.tile([P, 2, NIMG], f32r, tag="wg", name="wgt")
                nc.sync.dma_start(t[:], wgT_v[:, kt0:kt0 + 2, :])
                wg_tiles[gi] = t

            def emit_T(gi):
                a, kt0 = groups[gi]
                key = blk(gi)
                issue_xa(key)
                # prefetch xa for upcoming new blocks and wg for upcoming
                # glob groups
                seen = set()
                for fgi in range(gi + 1, min(gi + 2 * GPB, NG)):
                    fkey = blk(fgi)
                    if fkey not in seen:
                        seen.add(fkey)
                        issue_xa(fkey)
                    if groups[fgi][0] == 0 and fgi <= gi + GPB:
                        issue_wg(fgi)
                if a == 0:
                    issue_wg(gi)
                xa = xa_tiles[key]
                co = (kt0 - key[1] * KQ) * P
                pst = ps_t.tile([P, 512], fp32, tag="pst")
                for j in range(2):
                    nc.tensor.transpose(
                        pst[:, j * 256:j * 256 + Fc],
                        xa[:Fc, co + j * P:co + (j + 1) * P],
                        ident[:Fc, :Fc])
                xt = xtp.tile([P, 2, Fcp], f32r, tag="xt", name="xtt")
                pview = pst.rearrange("p (j c) -> p j c", j=2, c=256)[:, :, :Fc]
                if gi % 2 == 0:
                    nc.scalar.copy(xt[:, :, :Fc], pview)
                else:
                    nc.vector.tensor_copy(xt[:, :, :Fc], pview)
                xt_tiles[gi] = xt

            def emit_M(gi):
                a, kt0 = groups[gi]
                if a not in psout:
                    if a == 0:
                        psout[a] = [ps_mm.tile([P, 512], fp32, tag="mm",
                                               name=f"psg{n}") for n in range(2)]
                    else:
                        psout[a] = [ps_mm.tile([P, 512], fp32, tag="mm",
                                               name="psl")]
                    agent_done[a] = 0
                xt = xt_tiles.pop(gi)
                for j in range(2):
                    kt = kt0 + j
                    if a == 0:
                        wg = wg_tiles[gi]
                        for n in range(2):
                            nc.tensor.matmul(
                                psout[a][n][:Fc, :],
                                xt[:, j, :Fc],
                                wg[:, j, n * 512:(n + 1) * 512],
                                start=(kt == 0), stop=(kt == NKT - 1))
                    else:
                        nc.tensor.matmul(
                            psout[a][0][:Fc, :],
                            xt[:, j, :Fc],
                            wl_sb[:, kt, :],
                            start=(kt == 0), stop=(kt == NKT - 1))
                if a == 0:
                    wg_tiles.pop(gi)
                agent_done[a] += 1
                if agent_done[a] == GPA:
                    if a == 0:
                        gout = goutp.tile([Fcp, NIMG], fp32, tag="gout")
                        for n in range(2):
                            nc.vector.tensor_add(
                                gout[:Fc, n * 512:(n + 1) * 512],
                                psout[a][n][:Fc, :],
                                bg_bc[:Fc, n * 512:(n + 1) * 512])
                        nc.scalar.dma_start(glob_d.ap(), gout[:Fc, :])
                    else:
                        lout = outp.tile([Fcp, NATT], fp32, tag="lout")
                        nc.vector.tensor_add(lout[:Fc, :], psout[a][0][:Fc, :],
                                             bl_bc[:Fc, :])
                        nc.scalar.dma_start(loc_d.ap()[:, a - 1, :],
                                            lout[:Fc, :])
                    del psout[a]

            for i in range(NG + SKEW):
                if i < NG:
                    emit_T(i)
                if i >= SKEW:
                    emit_M(i - SKEW)
                pump_queue((i + 1) * n_q // (NG + SKEW))
            pump_queue(n_q)

    nc.compile()
    return nc


def _prep_weights(W_glob, b_glob, W_loc, b_loc, W_phys, b_phys, W_phys3, b_phys3):
    wglobT = np.ascontiguousarray(np.asarray(W_glob, np.float32).T)
    wlocT = np.ascontiguousarray(np.asarray(W_loc, np.float32).T)
    Wp = np.asarray(W_phys, np.float32)
    Wc = np.concatenate([Wp[:, :13], Wp[:, 13:]], axis=0)  # (512, 13)
    wcT = np.ascontiguousarray(Wc.T)  # (13, 512)
    w3T = np.ascontiguousarray(np.asarray(W_phys3, np.float32).T)
    return {
        "wglobT": wglobT, "wlocT": wlocT, "wcT": wcT, "w3T": w3T,
        "b_glob": np.asarray(b_glob, np.float32),
        "b_loc": np.asarray(b_loc, np.float32),
        "b_phys": np.asarray(b_phys, np.float32),
        "b_phys3": np.asarray(b_phys3, np.float32),
    }


def make_in_maps(x, world_coord_feat, **w):
    wmap = _prep_weights(**w)
    xf = np.asarray(x, np.float32).reshape(FTOT, A, NIN)
    cf = np.asarray(world_coord_feat, np.float32).reshape(FTOT, A, 13)
    in_maps = []
    for c in range(NCORES):
        sl = slice(c * F, (c + 1) * F)
        wcfT = np.ascontiguousarray(cf[sl].reshape(F * A, 13).T)
        in_maps.append({"x": np.ascontiguousarray(xf[sl]), "wcfT": wcfT, **wmap})
    return in_maps


def kernel(x, world_coord_feat, W_glob, b_glob, W_loc, b_loc,
           W_phys, b_phys, W_phys3, b_phys3):
    from concourse.bass_utils import run_bass_kernel_spmd

    if "nc" not in _CACHE:
        _CACHE["nc"] = build_module()
    nc = _CACHE["nc"]

    in_maps = make_in_maps(
        x, world_coord_feat,
        W_glob=W_glob, b_glob=b_glob, W_loc=W_loc, b_loc=b_loc,
        W_phys=W_phys, b_phys=b_phys, W_phys3=W_phys3, b_phys3=b_phys3)

    res = run_bass_kernel_spmd(nc, in_maps, list(range(NCORES)))
    _CACHE["last_results"] = res

    glob = np.concatenate([res.results[c]["glob"] for c in range(NCORES)], axis=0)
    loc = np.concatenate([res.results[c]["loc"] for c in range(NCORES)], axis=0)
    agent = np.concatenate([res.results[c]["agent"] for c in range(NCORES)], axis=0)
    return (glob.reshape(B, T, NIMG),
            loc.reshape(B, T, A - 1, NATT),
            agent.reshape(B, T, A, NPHYS))


# revision 13
# speedup vs baseline: 1.0397x; 1.0397x over previous
"""Trainium2 Bass kernel for nn_EmbedLayer (gnn_message_passing).

Computes, for x:(B,T,A,NIN), wcf:(B,T,A,13):
  glob  = x[:,:,0,:]  @ W_glob.T + b_glob            (B,T,1024)
  loc   = x[:,:,1:,:] @ W_loc.T  + b_loc             (B,T,19,512)
  h1    = relu(perm_feat @ W_phys.T + b_phys)        (B,T,190,256)
  phys  = h1 @ W_phys3.T + b_phys3
  agent = segment-max of phys over the 19 pairs containing each agent

Strategy: data-parallel over the 1000 (B,T) frames across 8 NeuronCores
(125 frames/core).  On each core:
 - loc/glob: X rows are PE-transposed into [K,128]-tiles (fp32), then fp32r
   matmuls with X^T stationary and W^T (pre-transposed on host) streamed,
   accumulating over K=4096 into PSUM [125,512].  The transpose->copy->matmul
   chain is software-pipelined (transposes run SKEW groups ahead of their
   matmuls) and the DMA-heavy glob agent is spread across the schedule.
 - pair branch: U = wcf @ [W1|W2]^T is computed feature-on-partition, the
   190-pair expansion is done with broadcast adds over the lexicographic
   run structure of the pairs, phys3 runs feature-on-partition, and the
   per-agent max uses the run structure before a final PE transpose back
   to row-major.
"""

import os
import numpy as np
from contextlib import ExitStack

B, T, A, NIN = 10, 100, 20, 4096
NIMG, NATT, NPHYS = 1024, 512, 256
NCORES = 8
FTOT = B * T
F = FTOT // NCORES  # 125 frames per core
NPAIR = 190
P = 128
NKT = NIN // P  # 32 k-tiles

_CACHE = {}


def make_chunks(fh):
    """Pair-run work units.  Run i covers pairs (i, j>i): fh*(19-i) columns.
    Big runs are split by frame-range so the SBUF tile stays small; small
    tail runs are grouped so every chunk has >=256 (even) matmul columns.
    Returns a list of (entries, cols) where entries = [(i, f0, fw), ...].
    """
    MAXC = 640
    chunks = []
    cur, cols = [], 0
    for i in range(A - 1):
        w = A - 1 - i
        rc = fh * w
        if rc >= 256:
            # split this run alone into frame-parts
            if cur and cols >= 256:
                chunks.append((cur, cols))
                cur, cols = [], 0
            nparts = -(-rc // MAXC)
            base = fh // nparts
            rem = fh - base * nparts
            f0 = 0
            for k in range(nparts):
                fw = base + (1 if k < rem else 0)
                cur.append((i, f0, fw))
                cols += fw * w
                f0 += fw
                if cols >= 256:
                    chunks.append((cur, cols))
                    cur, cols = [], 0
        else:
            cur.append((i, 0, fh))
            cols += rc
            if cols >= 256:
                chunks.append((cur, cols))
                cur, cols = [], 0
    if cur:
        if chunks and cols < 256:
            pi, pc = chunks[-1]
            chunks[-1] = (pi + cur, pc + cols)
        else:
            chunks.append((cur, cols))
    return chunks


def subsizes(cols):
    # fp32r matmul requires even moving/dest innermost counts
    assert cols % 2 == 0, cols
    n = -(-cols // 512)
    base = (cols // n) & ~1
    extra = cols - base * n
    assert extra % 2 == 0
    out = [base] * n
    k = 0
    while extra > 0:
        out[k] += 2
        extra -= 2
        k = (k + 1) % n
    return out


def suboffs(cols):
    szs = subsizes(cols)
    offs, o = [], 0
    for s in szs:
        offs.append(o)
        o += s
    return list(zip(offs, szs))


def build_module(Fc=F):
    import concourse.tile as tile
    import concourse.mybir as mybir
    from concourse import bacc
    from concourse.masks import make_identity

    fp32 = mybir.dt.float32
    f32r = mybir.dt.float32r
    AF = mybir.ActivationFunctionType
    AX = mybir.AxisListType

    nc = bacc.Bacc("TRN2", target_bir_lowering=False, debug=False)

    x_d = nc.dram_tensor("x", [Fc, A, NIN], f32r, kind="ExternalInput")
    wcfT_d = nc.dram_tensor("wcfT", [13, Fc * A], f32r, kind="ExternalInput")
    wgT_d = nc.dram_tensor("wglobT", [NIN, NIMG], f32r, kind="ExternalInput")
    wlT_d = nc.dram_tensor("wlocT", [NIN, NATT], f32r, kind="ExternalInput")
    wcT_d = nc.dram_tensor("wcT", [13, 2 * NPHYS], f32r, kind="ExternalInput")
    w3T_d = nc.dram_tensor("w3T", [NPHYS, NPHYS], f32r, kind="ExternalInput")
    bg_d = nc.dram_tensor("b_glob", [NIMG], fp32, kind="ExternalInput")
    bl_d = nc.dram_tensor("b_loc", [NATT], fp32, kind="ExternalInput")
    bp_d = nc.dram_tensor("b_phys", [NPHYS], fp32, kind="ExternalInput")
    b3_d = nc.dram_tensor("b_phys3", [NPHYS], fp32, kind="ExternalInput")

    identr_d = nc.dram_tensor("identr", [P, P], f32r, kind="ExternalInput")
    glob_d = nc.dram_tensor("glob", [Fc, NIMG], fp32, kind="ExternalOutput")
    loc_d = nc.dram_tensor("loc", [Fc, A - 1, NATT], fp32, kind="ExternalOutput")
    agent_d = nc.dram_tensor("agent", [Fc, A, NPHYS], fp32, kind="ExternalOutput")

    Fcp = min(Fc, P)
    halves = []
    f0 = 0
    for fh in (Fc - Fc // 2, Fc // 2):
        if fh:
            halves.append((f0, fh))
        f0 += fh
    fh_max = max(fh for _, fh in halves)
    ph_cols_max = max(cols + cols % 2
                      for _, fh in halves for _, cols in make_chunks(fh))

    KQ = 8            # k-tiles per xa block
    GPB = KQ // 2     # transpose groups per xa block
    BPA = NKT // KQ   # xa blocks per agent
    GPA = NKT // 2    # groups per agent
    SKEW = 3          # transpose groups emitted ahead of their matmuls

    with tile.TileContext(nc) as tc:
        with ExitStack() as ctx:
            const = ctx.enter_context(tc.tile_pool(name="const", bufs=1))
            wlp = ctx.enter_context(tc.tile_pool(name="wl", bufs=1))
            wgp = ctx.enter_context(tc.tile_pool(name="wg", bufs=3))
            xap = ctx.enter_context(tc.tile_pool(name="xa", bufs=5))
            xtp = ctx.enter_context(tc.tile_pool(name="xt", bufs=SKEW + 3))
            up = ctx.enter_context(tc.tile_pool(name="u", bufs=5))
            wcfp = ctx.enter_context(tc.tile_pool(name="wcf", bufs=1))
            php = ctx.enter_context(tc.tile_pool(name="ph", bufs=7))
            amp = ctx.enter_context(tc.tile_pool(name="am", bufs=3))
            tmpp = ctx.enter_context(tc.tile_pool(name="tmp", bufs=2))
            outp = ctx.enter_context(tc.tile_pool(name="out", bufs=2))
            goutp = ctx.enter_context(tc.tile_pool(name="gout", bufs=1))
            aoutp = ctx.enter_context(tc.tile_pool(name="aout", bufs=2))

            ps_t = ctx.enter_context(tc.tile_pool(name="ps_t", bufs=2, space="PSUM"))
            ps_mm = ctx.enter_context(tc.tile_pool(name="ps_mm", bufs=3, space="PSUM"))
            ps_ph = ctx.enter_context(tc.tile_pool(name="ps_ph", bufs=2, space="PSUM"))

            ident = const.tile([P, P], fp32)
            make_identity(nc, ident[:])
            identr = const.tile([P, P], f32r)
            nc.sync.dma_start(identr[:], identr_d.ap())

            # ---- small constants first (fast DMAs) ----
            wcT_sb = const.tile([13, 2 * NPHYS], f32r)
            nc.sync.dma_start(wcT_sb[:], wcT_d.ap())
            w3_sb = const.tile([P, 2, NPHYS], f32r)
            nc.sync.dma_start(w3_sb[:], w3T_d.ap().rearrange("(kt p) n -> p kt n", p=P))
            bp_sb = const.tile([P, 2], fp32)
            nc.sync.dma_start(bp_sb[:], bp_d.ap().rearrange("(b p) -> p b", p=P))
            b3_sb = const.tile([P, 2], fp32)
            nc.sync.dma_start(b3_sb[:], b3_d.ap().rearrange("(b p) -> p b", p=P))
            bl_bc = const.tile([P, NATT], fp32)
            nc.sync.dma_start(bl_bc[:1, :], bl_d.ap()[None, :])
            nc.gpsimd.partition_broadcast(bl_bc[:Fcp, :], bl_bc[:1, :])
            bg_bc = const.tile([P, NIMG], fp32)
            nc.sync.dma_start(bg_bc[:1, :], bg_d.ap()[None, :])
            nc.gpsimd.partition_broadcast(bg_bc[:Fcp, :], bg_bc[:1, :])

            # loc weights resident, loaded in 4 chunks on the SWDGE queue so
            # they don't serialize the SP HWDGE stream of xa loads
            wl_sb = wlp.tile([P, NKT, NATT], f32r)
            wlT_v = wlT_d.ap().rearrange("(kt p) n -> p kt n", p=P)
            for c in range(4):
                nc.gpsimd.dma_start(wl_sb[:, c * 8:(c + 1) * 8, :],
                                    wlT_v[:, c * 8:(c + 1) * 8, :])

            wgT_v = wgT_d.ap().rearrange("(kt p) n -> p kt n", p=P)
            agrows = agent_d.ap().rearrange("f a n -> (f a) n")

            u_tiles = {}
            am_tiles = {}

            def emit_u(h):
                hf0, fh = halves[h]
                wcf_sb = wcfp.tile([13, fh_max * A], f32r, tag="wcf")
                nc.sync.dma_start(wcf_sb[:, :fh * A],
                                  wcfT_d.ap()[:, hf0 * A:(hf0 + fh) * A])
                uts = []
                for m in range(4):
                    ut = up.tile([P, fh_max * A], fp32, tag="u")
                    uts.append(ut)
                    for so, sw in suboffs(fh * A):
                        pu = ps_ph.tile([P, 512], fp32, tag="psph")
                        nc.tensor.matmul(pu[:, :sw],
                                         wcT_sb[:, m * P:(m + 1) * P],
                                         wcf_sb[:, so:so + sw],
                                         start=True, stop=True)
                        if m < 2:
                            nc.scalar.activation(ut[:, so:so + sw], pu[:, :sw],
                                                 AF.Identity, bias=bp_sb[:, m:m + 1])
                        else:
                            nc.scalar.copy(ut[:, so:so + sw], pu[:, :sw])
                u_tiles[h] = uts
                amb = []
                for b in range(2):
                    am = amp.tile([P, fh_max * A], fp32, tag="am")
                    nc.gpsimd.memset(am[:], -1e30)
                    amb.append(am)
                am_tiles[h] = amb

            def emit_phys_chunk(h, entries, cols):
                hf0, fh = halves[h]
                uts = u_tiles[h]
                cols_pad = cols + (cols % 2)
                offs = [0]
                for (i, e0, ew) in entries:
                    offs.append(offs[-1] + ew * (A - 1 - i))
                h1s = []
                for b in range(2):
                    h1pre = php.tile([P, ph_cols_max], fp32, tag="ph")
                    uv0 = uts[b].rearrange("p (f a) -> p f a", a=A)
                    uv2 = uts[2 + b].rearrange("p (f a) -> p f a", a=A)
                    for r, (i, e0, ew) in enumerate(entries):
                        w = A - 1 - i
                        ov = h1pre[:, offs[r]:offs[r + 1]].rearrange(
                            "p (f q) -> p f q", q=w)
                        nc.gpsimd.tensor_add(
                            ov,
                            uv0[:, e0:e0 + ew, i:i + 1].to_broadcast((P, ew, w)),
                            uv2[:, e0:e0 + ew, i + 1:A])
                    if cols_pad != cols:
                        nc.gpsimd.memset(h1pre[:, cols:cols_pad], 0.0)
                    h1 = php.tile([P, ph_cols_max], f32r, tag="ph", name="h1")
                    nc.scalar.activation(h1[:, :cols_pad], h1pre[:, :cols_pad],
                                         AF.Relu)
                    h1s.append(h1)
                for b in range(2):
                    py = php.tile([P, ph_cols_max], fp32, tag="ph")
                    for so, sw in suboffs(cols_pad):
                        pp = ps_ph.tile([P, 512], fp32, tag="psph")
                        for kt in range(2):
                            nc.tensor.matmul(
                                pp[:, :sw],
                                w3_sb[:, kt, b * P:(b + 1) * P],
                                h1s[kt][:, so:so + sw],
                                start=(kt == 0), stop=(kt == 1))
                        nc.scalar.activation(py[:, so:so + sw], pp[:, :sw],
                                             AF.Identity, bias=b3_sb[:, b:b + 1])
                    amv = am_tiles[h][b].rearrange("p (f a) -> p f a", a=A)
                    for r, (i, e0, ew) in enumerate(entries):
                        w = A - 1 - i
                        pv = py[:, offs[r]:offs[r + 1]].rearrange(
                            "p (f q) -> p f q", q=w)
                        # pairs (i, j) update agents j = i+1..19
                        nc.vector.tensor_max(amv[:, e0:e0 + ew, i + 1:A],
                                             amv[:, e0:e0 + ew, i + 1:A], pv)
                        # max over the run updates agent i
                        tmp = tmpp.tile([P, P], fp32, tag="tmp")
                        nc.vector.reduce_max(tmp[:, :ew], pv, axis=AX.X)
                        nc.vector.tensor_max(amv[:, e0:e0 + ew, i],
                                             amv[:, e0:e0 + ew, i], tmp[:, :ew])

            def emit_agent_out(h):
                hf0, fh = halves[h]
                rows = fh * A
                r0 = 0
                while r0 < rows:
                    cw = min(125, rows - r0)
                    pst = ps_t.tile([P, 512], fp32, tag="pst")
                    for b in range(2):
                        nc.tensor.transpose(pst[:cw, b * 256:b * 256 + P],
                                            am_tiles[h][b][:, r0:r0 + cw],
                                            ident[:, :])
                    aout = aoutp.tile([125, NPHYS], fp32, tag="aout")
                    nc.scalar.copy(
                        aout[:cw, :],
                        pst.rearrange("p (b c) -> p b c", b=2, c=256)[:cw, :, :P])
                    nc.scalar.dma_start(agrows[hf0 * A + r0:hf0 * A + r0 + cw, :],
                                        aout[:cw, :])
                    r0 += cw

            # phys work queue: halves strictly sequential
            queue = []
            emit_u(0)
            for entries, cols in make_chunks(halves[0][1]):
                queue.append((emit_phys_chunk, (0, entries, cols)))

            def h0_to_h1():
                emit_agent_out(0)
                if len(halves) > 1:
                    emit_u(1)
            queue.append((h0_to_h1, ()))
            if len(halves) > 1:
                for entries, cols in make_chunks(halves[1][1]):
                    queue.append((emit_phys_chunk, (1, entries, cols)))
                queue.append((emit_agent_out, (1,)))
            n_q = len(queue)
            q_emitted = [0]

            def pump_queue(target):
                while q_emitted[0] < min(target, n_q):
                    fn, args = queue[q_emitted[0]]
                    fn(*args)
                    q_emitted[0] += 1

            # ---- loc/glob pipeline over a flat group list ----
            # group = (a, kt0): 2 k-tile transposes + 2 matmuls.
            # agent 0 (glob, 16MB of streamed weights) is spread through the
            # schedule one group per ~19 slots.
            loc_groups = [(a, g * 2) for a in range(1, A) for g in range(GPA)]
            glob_groups = [(0, g * 2) for g in range(GPA)]
            groups = []
            li = 0
            step = max(1, len(loc_groups) // len(glob_groups))
            for k, gg in enumerate(glob_groups):
                groups.append(gg)
                groups.extend(loc_groups[li:li + step])
                li += step
            groups.extend(loc_groups[li:])
            NG = len(groups)

            def blk(gi):
                a, kt0 = groups[gi]
                return (a, kt0 // KQ)

            xa_tiles = {}
            xa_issued = set()
            xt_tiles = {}
            wg_tiles = {}
            wg_issued = set()
            psout = {}
            agent_done = {}

            def issue_xa(key):
                if key in xa_issued:
                    return
                xa_issued.add(key)
                a, q = key
                t = xap.tile([Fcp + 1, KQ * P], f32r, tag="xa", name="xat")
                nc.sync.dma_start(t[:Fc, :],
                                  x_d.ap()[:, a, q * KQ * P:(q + 1) * KQ * P])
                xa_tiles[key] = t

            def issue_wg(gi):
                if gi in wg_issued:
                    return
                wg_issued.add(gi)
                a, kt0 = groups[gi]
                t = wgp.tile([P, 2, NIMG], f32r, tag="wg", name="wgt")
                nc.sync.dma_start(t[:], wgT_v[:, kt0:kt0 + 2, :])
                wg_tiles[gi] = t

            def emit_T(gi):
                a, kt0 = groups[gi]
                key = blk(gi)
                issue_xa(key)
                # prefetch xa for upcoming new blocks and wg for upcoming
                # glob groups
                seen = set()
                for fgi in range(gi + 1, min(gi + 2 * GPB, NG)):
                    fkey = blk(fgi)
                    if fkey not in seen:
                        seen.add(fkey)
                        issue_xa(fkey)
                    if groups[fgi][0] == 0 and fgi <= gi + GPB:
                        issue_wg(fgi)
                if a == 0:
                    issue_wg(gi)
                xa = xa_tiles[key]
                co = (kt0 - key[1] * KQ) * P
                Fe = Fc + (Fc % 2)
                pst = ps_t.tile([P, 512], f32r, tag="pst")
                for j in range(2):
                    nc.tensor.transpose(
                        pst[:, j * 256:j * 256 + Fe],
                        xa[:Fe, co + j * P:co + (j + 1) * P],
                        identr[:Fe, :Fe])
                xt = xtp.tile([P, 2, Fcp], f32r, tag="xt", name="xtt")
                pview = pst.rearrange("p (j c) -> p j c", j=2, c=256)[:, :, :Fc]
                if gi % 2 == 0:
                    nc.scalar.copy(xt[:, :, :Fc], pview)
                else:
                    nc.vector.tensor_copy(xt[:, :, :Fc], pview)
                xt_tiles[gi] = xt

            def emit_M(gi):
                a, kt0 = groups[gi]
                if a not in psout:
                    if a == 0:
                        psout[a] = [ps_mm.tile([P, 512], fp32, tag="mm",
                                               name=f"psg{n}") for n in range(2)]
                    else:
                        psout[a] = [ps_mm.tile([P, 512], fp32, tag="mm",
                                               name="psl")]
                    agent_done[a] = 0
                xt = xt_tiles.pop(gi)
                for j in range(2):
                    kt = kt0 + j
                    if a == 0:
                        wg = wg_tiles[gi]
                        for n in range(2):
                            nc.tensor.matmul(
                                psout[a][n][:Fc, :],
                                xt[:, j, :Fc],
                                wg[:, j, n * 512:(n + 1) * 512],
                                start=(kt == 0), stop=(kt == NKT - 1))
                    else:
                        nc.tensor.matmul(
                            psout[a][0][:Fc, :],
                            xt[:, j, :Fc],
                            wl_sb[:, kt, :],
                            start=(kt == 0), stop=(kt == NKT - 1))
                if a == 0:
                    wg_tiles.pop(gi)
                agent_done[a] += 1
                if agent_done[a] == GPA:
                    if a == 0:
                        gout = goutp.tile([Fcp, NIMG], fp32, tag="gout")
                        for n in range(2):
                            nc.vector.tensor_add(
                                gout[:Fc, n * 512:(n + 1) * 512],
                                psout[a][n][:Fc, :],
                                bg_bc[:Fc, n * 512:(n + 1) * 512])
                        nc.scalar.dma_start(glob_d.ap(), gout[:Fc, :])
                    else:
                        lout = outp.tile([Fcp, NATT], fp32, tag="lout")
                        nc.vector.tensor_add(lout[:Fc, :], psout[a][0][:Fc, :],
                                             bl_bc[:Fc, :])
                        nc.scalar.dma_start(loc_d.ap()[:, a - 1, :],
                                            lout[:Fc, :])
                    del psout[a]

            for i in range(NG + SKEW):
                if i < NG:
                    emit_T(i)
                if i >= SKEW:
                    emit_M(i - SKEW)
                pump_queue((i + 1) * n_q // (NG + SKEW))
            pump_queue(n_q)

    nc.compile()
    return nc


def _prep_weights(W_glob, b_glob, W_loc, b_loc, W_phys, b_phys, W_phys3, b_phys3):
    wglobT = np.ascontiguousarray(np.asarray(W_glob, np.float32).T)
    wlocT = np.ascontiguousarray(np.asarray(W_loc, np.float32).T)
    Wp = np.asarray(W_phys, np.float32)
    Wc = np.concatenate([Wp[:, :13], Wp[:, 13:]], axis=0)  # (512, 13)
    wcT = np.ascontiguousarray(Wc.T)  # (13, 512)
    w3T = np.ascontiguousarray(np.asarray(W_phys3, np.float32).T)
    return {
        "wglobT": wglobT, "wlocT": wlocT, "wcT": wcT, "w3T": w3T,
        "b_glob": np.asarray(b_glob, np.float32),
        "b_loc": np.asarray(b_loc, np.float32),
        "b_phys": np.asarray(b_phys, np.float32),
        "b_phys3": np.asarray(b_phys3, np.float32),
    }


def make_in_maps(x, world_coord_feat, **w):
    wmap = _prep_weights(**w)
    xf = np.asarray(x, np.float32).reshape(FTOT, A, NIN)
    cf = np.asarray(world_coord_feat, np.float32).reshape(FTOT, A, 13)
    in_maps = []
    for c in range(NCORES):
        sl = slice(c * F, (c + 1) * F)
        wcfT = np.ascontiguousarray(cf[sl].reshape(F * A, 13).T)
        in_maps.append({"x": np.ascontiguousarray(xf[sl]), "wcfT": wcfT,
                        "identr": np.eye(P, dtype=np.float32), **wmap})
    return in_maps


def kernel(x, world_coord_feat, W_glob, b_glob, W_loc, b_loc,
           W_phys, b_phys, W_phys3, b_phys3):
    from concourse.bass_utils import run_bass_kernel_spmd

    if "nc" not in _CACHE:
        _CACHE["nc"] = build_module()
    nc = _CACHE["nc"]

    in_maps = make_in_maps(
        x, world_coord_feat,
        W_glob=W_glob, b_glob=b_glob, W_loc=W_loc, b_loc=b_loc,
        W_phys=W_phys, b_phys=b_phys, W_phys3=W_phys3, b_phys3=b_phys3)

    res = run_bass_kernel_spmd(nc, in_maps, list(range(NCORES)))
    _CACHE["last_results"] = res

    glob = np.concatenate([res.results[c]["glob"] for c in range(NCORES)], axis=0)
    loc = np.concatenate([res.results[c]["loc"] for c in range(NCORES)], axis=0)
    agent = np.concatenate([res.results[c]["agent"] for c in range(NCORES)], axis=0)
    return (glob.reshape(B, T, NIMG),
            loc.reshape(B, T, A - 1, NATT),
            agent.reshape(B, T, A, NPHYS))
